# revision 23
# baseline (speedup 1.0000x reference)
"""Trainium2 Bass kernel for the autoregressive GRU decode head.

Problem: context = mean over zones of encoded_features[:, -1]  -> (B, D)
then 12 autoregressive steps of a 2-layer GRU (H=256) + linear projection
to N=256 zones.  B=1024, data-parallel across 8 NeuronCores (128 batch each).

Key structure (per core, feature-major / "transposed" activations):
  actT (128p, 4 slots, 128) bf16 : [h0 c0, h0 c1, h1 c0, h1 c1]
       slot holds h[b, c*128 + p] at [p, b]    (c = chunk of the 256-dim)
  Gate tensors (PSUM) use layout [p, c*128 + b].
  Matmuls: out(gate_chunk, B) = lhsT.T @ rhs, lhsT = W^T tile (K<=128, M=128),
  rhs = actT slot (K=128, B=128), K-chunks accumulated in PSUM.
  The prediction feedback is algebraically folded into layer 0's weights:
  W_pred @ (W_out h1 + b_out) = (W_pred W_out) h1 + W_pred b_out, so the
  recurrence never materializes pred; pred is computed batch-major only for
  the DRAM output.
  ALL gate biases are injected into PSUM as K=1 matmuls (bias-row x ones),
  so each gate chain is: one wide sigmoid ACT + single-instruction DVE ops.
  Matmul emission is software-pipelined across steps: the h0-dependent
  matmul groups for step t+1 are emitted right after chain(0) of step t, so
  each sigmoid only waits on the 8 matmuls that depend on the newest hidden.
The encoded_features slice is streamed as bf16 (host-converted); the zone
mean is a pairwise add tree split across DVE and GpSimd.
"""

import sys

for _p in ("/opt/trn_rl_repo",):
    if _p not in sys.path:
        sys.path.insert(0, _p)

import numpy as np
import ml_dtypes

import concourse.bass as bass
import concourse.tile as tile
from concourse import mybir
from concourse.vector_clock import ScopedClock

BF16 = ml_dtypes.bfloat16

B, T, NZ, D = 1024, 8, 256, 256
H = 256
STEPS = 12
N_CORES = 8
PC = B // N_CORES  # 128 batch per core

F32 = mybir.dt.float32
BF = mybir.dt.bfloat16
AF = mybir.ActivationFunctionType
OP = mybir.AluOpType

# bias-row column layout in the (1, NBROW*128) bf16 bias-rows tensor:
# value[row*128 + j] is the bias for gate index (chunk c) at position j
_RZ0 = 0                      # 12 steps * 4 chunks
_IN0 = _RZ0 + STEPS * 4      # 12 steps * 2 chunks
_HN0 = _IN0 + STEPS * 2      # 2
_RZ1 = _HN0 + 2               # 4
_IN1 = _RZ1 + 4               # 2
_HN1 = _IN1 + 2               # 2
_BOUT = _HN1 + 2              # 2 (b_out folded into the wout PSUM group)
NBROW = _BOUT + 2             # 84


def _install_tile_drain_patch():
    """walrus (CoreV3) rejects >1 sync wait on the tail drain; spill extras
    onto preceding sync nops."""
    if getattr(tile.TileContext, "_drain_patch_installed", False):
        return

    def _patched(self, tick_clock, wait_clock):
        nc = self.nc
        bb = nc.cur_bb.bb
        drain_bi = nc.sync.drain()
        drain_inst = drain_bi.ins
        wait_clock.add_sem_waits(
            drain_inst, ScopedClock({None: tick_clock.global_clock})
        )
        w = drain_inst.sync_info.on_wait if drain_inst.sync_info else None
        maxw = 1
        if w and len(w) > maxw:
            extra = list(w[maxw:])
            drain_inst.sync_info.on_wait = list(w[:maxw])
            idx = bb.instructions.index(drain_inst)
            nops = []
            for i in range(0, len(extra), maxw):
                nop_bi = nc.sync.nop()
                nop = nop_bi.ins
                si = nop.sync_info
                nop.sync_info = mybir.SyncInfo(
                    on_wait=extra[i : i + maxw],
                    on_update=(si.on_update if si else []),
                )
                bb.instructions.remove(nop)
                nops.append(nop)
            bb.instructions[idx:idx] = nops
        nc.all_engine_barrier()
        popped = nc._tile_sem_poison_stack.pop()
        assert popped is self._sem_poison
        nc.clear_and_free_semaphores(list(self.sems.allocated().values()))
        nc.all_engine_barrier()

    tile.TileContext._drain_and_barrier = _patched
    tile.TileContext._drain_patch_installed = True


def _split_waits(nc, maxw=1):
    """This walrus build rejects instructions carrying more than ~1 sem
    wait; spill extra waits onto same-engine nops placed just before."""
    for bb in nc.main_func.blocks:
        new_list = []
        for inst in bb.instructions:
            si = inst.sync_info
            w = list(si.on_wait) if si and si.on_wait else []
            if len(w) > maxw:
                keep = w[len(w) - maxw:]
                extra = w[: len(w) - maxw]
                si.on_wait = keep
                for i in range(0, len(extra), maxw):
                    nop = mybir.InstNoOp(
                        name=f"{inst.name}-sw{i}", ins=[], outs=[]
                    )
                    nop.engine = inst.engine
                    nop.sync_info = mybir.SyncInfo(
                        on_wait=extra[i : i + maxw], on_update=[]
                    )
                    nc.register_instruction(nop)
                    new_list.append(nop)
            new_list.append(inst)
        bb.instructions[:] = new_list


class _Group:
    """Tracks start/stop flags for a PSUM accumulation group whose matmuls
    are emitted in several program-order batches."""

    def __init__(self, total):
        self.total = total
        self.emitted = 0

    def flags(self):
        start = self.emitted == 0
        self.emitted += 1
        return start, self.emitted == self.total


def build_kernel(nsteps=12):
    """Build the per-core Bass graph (SPMD: same graph on all 8 cores)."""
    _install_tile_drain_patch()
    nc = bass.Bass()

    enc = nc.declare_dram_parameter("enc", [PC, NZ, D], BF, isOutput=False)
    # all matmul weights pre-arranged on host into one contiguous wall
    # [128 part, 6656] so phase 1 needs only ONE weight DMA (the tile
    # framework has an 8-semaphore DMA pool; more DMAs = false serialization)
    WOFF = {"wrz0": (0, 4, 512), "win0": (2048, 2, 256),
            "whn0": (2560, 2, 256), "wrz1": (3072, 4, 512),
            "win1": (5120, 2, 256), "whn1": (5632, 2, 256),
            "wout": (6144, 2, 256)}
    wall = nc.declare_dram_parameter("wall", [128, 6656], BF, isOutput=False)
    brows = nc.declare_dram_parameter("brows", [1, NBROW * 128], BF,
                                      isOutput=False)
    out = nc.declare_dram_parameter("out", [PC, STEPS, NZ], BF, isOutput=True)

    with tile.TileContext(nc) as tc:
        with (
            tc.tile_pool(name="consts", bufs=1) as consts,
            tc.tile_pool(name="state", bufs=1) as state,
            tc.tile_pool(name="enc_pool", bufs=1) as enc_pool,
            tc.tile_pool(name="gates", bufs=2) as gates,
            tc.tile_pool(name="ostage", bufs=4) as ostage,
            tc.tile_pool(name="psum", bufs=1, space="PSUM") as psum,
        ):
            # ---- weight / bias loads on the scalar HWDGE queue ----
            # (emitted first so they grab fresh DMA semaphores)
            wall_sb = consts.tile([128, 6656], BF, tag="wall")
            nc.scalar.dma_start(wall_sb[:], wall[:])
            w_sb = {
                name: wall_sb[:, off : off + kc * mdim].rearrange(
                    "p (k m) -> p k m", k=kc)
                for name, (off, kc, mdim) in WOFF.items()
            }
            brow_sb = consts.tile([1, NBROW * 128], BF, tag="brow")
            nc.scalar.dma_start(brow_sb[:], brows[:])

            # ---- phase 1 enc DMA: one queue, 64-zone chunks ----
            # Per-queue DMA throughput is descriptor-rate bound (~13M/s):
            # 64-zone chunks give 32KB descriptors (~420 GB/s, the cap).
            # Multi-queue splits measured SLOWER (HBM thrash).  All chunks
            # resident - no buffer-reuse stalls.
            ZCHS = [64, 64, 64, 48, 16]
            NCH = len(ZCHS)
            e_tiles = []
            z0 = 0
            for ci, zch in enumerate(ZCHS):
                e_sb = enc_pool.tile([128, zch * D], BF, tag=f"echunk{ci}")
                nc.sync.dma_start(e_sb[:], enc[:, z0 : z0 + zch, :])
                e_tiles.append(e_sb)
                z0 += zch
            ones_row = consts.tile([1, 128], BF, tag="ones")
            nc.gpsimd.memset(ones_row[:], 1.0)

            identity = consts.tile([128, 128], F32, tag="ident")
            nc.gpsimd.memset(identity[:], 0.0)
            nc.gpsimd.affine_select(
                out=identity[:],
                in_=identity[:],
                compare_op=OP.not_equal,
                fill=1.0,
                base=0,
                pattern=[[-1, 128]],
                channel_multiplier=1,
            )
            # prewarm the sigmoid/tanh ACT table during phase 1
            warm = consts.tile([128, 1], F32, tag="warm")
            nc.scalar.activation(warm[:], identity[:, 0:1], AF.Sigmoid)

            # ---- phase 1: zone-mean tree per chunk, split DVE / GpSimd ----
            # GpSimd TT measures ~2.8 ns/elem vs DVE ~0.55: give it the
            # throughput-balanced ~1/6 share, and only on levels big
            # enough to amortize its fixed overhead.
            GSHARE = False
            partials = state.tile([128, NCH, D], F32, tag="partials")
            for i in range(NCH):
                e_sb = e_tiles[i]
                w = ZCHS[i] * D
                while w > 8 * D:
                    h = w // 2
                    fd = (h - h // 6) // D * D if (GSHARE and h >= 4096) else h
                    nc.vector.tensor_tensor(
                        e_sb[:, 0:fd], e_sb[:, 0:fd], e_sb[:, h : h + fd],
                        OP.add,
                    )
                    if fd < h:
                        nc.gpsimd.tensor_tensor(
                            e_sb[:, fd:h], e_sb[:, fd:h], e_sb[:, h + fd : w],
                            OP.add,
                        )
                    w = h
                # bf16 tail keeps the DVE 2x-mode (f32 only at the end)
                tmp = gates.tile([128, 4 * D], BF, tag="redtail")
                h = w // 2
                nc.vector.tensor_tensor(
                    tmp[:, 0:h], e_sb[:, 0:h], e_sb[:, h:w], OP.add
                )
                w = h
                while w > 2 * D:
                    if (w // 2) % D == 0:
                        h = w // 2
                        nc.vector.tensor_tensor(
                            tmp[:, 0:h], tmp[:, 0:h], tmp[:, h:w], OP.add
                        )
                    else:  # peel one D-block (w = 3*2^k*D cases)
                        h = w - D
                        nc.vector.tensor_tensor(
                            tmp[:, h - D : h], tmp[:, h - D : h], tmp[:, h:w],
                            OP.add,
                        )
                    w = h
                nc.vector.tensor_tensor(
                    partials[:, i, :], tmp[:, 0:D], tmp[:, D : 2 * D], OP.add
                )
                if i:
                    nc.vector.tensor_tensor(
                        partials[:, 0, :], partials[:, 0, :], partials[:, i, :],
                        OP.add,
                    )
            ctx = partials[:, 0, :]

            # ---- state: actT slots [h0c0, h0c1, h1c0, h1c1] ----
            actT = state.tile([128, 4, 128], BF, tag="actT")
            for c in range(2):
                ctps = psum.tile([128, 256], F32, tag="r0", bufs=1)
                nc.tensor.transpose(
                    ctps[:, 0:128], ctx[:, c * 128 : (c + 1) * 128],
                    identity[:]
                )
                nc.scalar.activation(actT[:, c, :], ctps[:, 0:128], AF.Copy,
                                     scale=1.0 / NZ)
                nc.scalar.activation(actT[:, 2 + c, :], ctps[:, 0:128],
                                     AF.Copy, scale=1.0 / NZ)

            def bias_mms(g, grp, base, ms):
                # inject bias rows into PSUM: out[c*128+p, b] += brow[p] * 1
                for mi, m in enumerate(ms):
                    st, sp = grp.flags()
                    nc.tensor.matmul(
                        g[:, mi * 128 : (mi + 1) * 128],
                        brow_sb[0:1, (base + m) * 128 : (base + m + 1) * 128],
                        ones_row[:], start=st, stop=sp,
                    )

            def gate_mms(g, grp, w_t, kis, slots, ms):
                for mi, m in enumerate(ms):
                    dst = g[:, mi * 128 : (mi + 1) * 128]
                    for ki, slot in zip(kis, slots):
                        st, sp = grp.flags()
                        nc.tensor.matmul(
                            dst, w_t[:, ki, m * 128 : (m + 1) * 128],
                            actT[:, slot, :], start=st, stop=sp,
                        )

            def chain(layer, g_r, g_z, g_hn, g_in):
                # all biases are already accumulated into the PSUM groups.
                # r and z are SEPARATE tiles so each sigmoid waits only its
                # own 4 matmuls (tile-granular dependency tracking).
                h_sl = (0, 1) if layer == 0 else (2, 3)
                s_ = gates.tile([128, 512], BF, tag=f"S{layer}")
                nc.scalar.activation(s_[:, 0:256], g_r[:], AF.Sigmoid)
                nc.scalar.activation(s_[:, 256:512], g_z[:], AF.Sigmoid)
                t_ = gates.tile([128, 256], F32, tag=f"tt{layer}")
                nc.vector.tensor_tensor(t_[:], g_hn[:], s_[:, 0:256], OP.mult)
                # v = g_in + t  (in-place accumulate into the PSUM tile)
                nc.vector.tensor_tensor(g_in[:], g_in[:], t_[:], OP.add)
                hv = actT[:, h_sl[0] : h_sl[0] + 2, :].rearrange(
                    "p a b -> p (a b)")
                # zm1 = z - 1 off the spine, so q_ is a cheap TT (not STT)
                zm1 = gates.tile([128, 256], BF, tag=f"zm{layer}")
                nc.vector.tensor_scalar_add(zm1[:], s_[:, 256:512], -1.0)
                c_ = gates.tile([128, 256], BF, tag=f"cc{layer}")
                nc.vector.tensor_tensor(c_[:], s_[:, 256:512], hv, OP.mult)
                n_ = gates.tile([128, 256], BF, tag=f"nn{layer}")
                nc.scalar.activation(n_[:], g_in[:], AF.Tanh)
                q_ = gates.tile([128, 256], BF, tag=f"qq{layer}")
                nc.vector.tensor_tensor(q_[:], zm1[:], n_[:], OP.mult)
                nc.vector.tensor_tensor(hv, c_[:], q_[:], OP.subtract)

            def emit_wout(t):
                # output projection, batch-major (off the recurrence)
                g_pb = psum.tile([128, 256], F32, tag="predB", bufs=1)
                nc.tensor.matmul(
                    g_pb[:], ones_row[:],
                    brow_sb[0:1, _BOUT * 128 : _BOUT * 128 + 256],
                    start=True, stop=False,
                )
                for ki, slot in ((0, 2), (1, 3)):
                    nc.tensor.matmul(
                        g_pb[:], actT[:, slot, :], w_sb["wout"][:, ki, :],
                        start=False, stop=(ki == 1),
                    )
                # stage on the scalar engine (GpSimd cannot read PSUM;
                # a DVE placement would head-of-line block the recurrence)
                o_ = ostage.tile([128, 256], BF, tag="ost")
                nc.scalar.activation(o_[:], g_pb[:], AF.Copy)
                nc.sync.dma_start(out[:, t, :], o_[:])

            def start_l0_groups(t):
                """Emit the h0(t-1)-dependent parts of step t's layer-0
                groups (bias rows + hidden matmuls).  The fold (h1) parts
                complete the groups at the top of step t."""
                g_r0 = psum.tile([128, 256], F32, tag="r0", bufs=1)
                g_z0 = psum.tile([128, 256], F32, tag="z0", bufs=1)
                # hn0 and in0 share one PSUM bank (one accumulation group)
                g_aux0 = psum.tile([128, 512], F32, tag="aux0", bufs=1)
                g_hn0 = g_aux0[:, 0:256]
                g_in0 = g_aux0[:, 256:512]
                gr0 = _Group(2 + 4 + (4 if t > 0 else 0))
                gz0 = _Group(2 + 4 + (4 if t > 0 else 0))
                gaux0 = _Group(2 + 4 + 2 + (4 if t > 0 else 0))
                bias_mms(g_r0, gr0, _RZ0 + t * 4, (0, 1))
                gate_mms(g_r0, gr0, w_sb["wrz0"], (2, 3), (0, 1), (0, 1))
                bias_mms(g_z0, gz0, _RZ0 + t * 4, (2, 3))
                gate_mms(g_z0, gz0, w_sb["wrz0"], (2, 3), (0, 1), (2, 3))
                bias_mms(g_hn0, gaux0, _HN0, (0, 1))
                gate_mms(g_hn0, gaux0, w_sb["whn0"], (0, 1), (0, 1), (0, 1))
                bias_mms(g_in0, gaux0, _IN0 + t * 2, (0, 1))
                return g_r0, gr0, g_z0, gz0, g_hn0, g_in0, gaux0

            # ---- phase 2: 12 decode steps, software-pipelined emission ----
            pend0 = start_l0_groups(0)  # h0(-1) = ctx, no fold at t=0

            for t in range(nsteps):
                g_r0, gr0, g_z0, gz0, g_hn0, g_in0, gaux0 = pend0

                # --- phase A: needs h1(t-1) ---
                if t > 0:
                    # pred feedback folded onto h1(t-1); r first (spine)
                    gate_mms(g_r0, gr0, w_sb["wrz0"], (0, 1), (2, 3), (0, 1))
                    gate_mms(g_z0, gz0, w_sb["wrz0"], (0, 1), (2, 3), (2, 3))
                    gate_mms(g_in0, gaux0, w_sb["win0"], (0, 1), (2, 3),
                             (0, 1))
                # layer 1 own-hidden parts (need h1(t-1) only)
                g_r1 = psum.tile([128, 256], F32, tag="r1", bufs=1)
                g_z1 = psum.tile([128, 256], F32, tag="z1", bufs=1)
                # hn1 and in1 share one PSUM bank (one accumulation group)
                g_aux1 = psum.tile([128, 512], F32, tag="aux1", bufs=1)
                g_hn1 = g_aux1[:, 0:256]
                g_in1 = g_aux1[:, 256:512]
                gr1 = _Group(2 + 4 + 4)
                gz1 = _Group(2 + 4 + 4)
                gaux1 = _Group(2 + 4 + 2 + 4)
                bias_mms(g_r1, gr1, _RZ1, (0, 1))
                gate_mms(g_r1, gr1, w_sb["wrz1"], (2, 3), (2, 3), (0, 1))
                bias_mms(g_z1, gz1, _RZ1, (2, 3))
                gate_mms(g_z1, gz1, w_sb["wrz1"], (2, 3), (2, 3), (2, 3))
                bias_mms(g_hn1, gaux1, _HN1, (0, 1))
                gate_mms(g_hn1, gaux1, w_sb["whn1"], (0, 1), (2, 3), (0, 1))
                if t > 0:
                    emit_wout(t - 1)

                chain(0, g_r0, g_z0, g_hn0, g_in0)  # -> h0(t)

                # --- phase B: needs h0(t) ---
                gate_mms(g_r1, gr1, w_sb["wrz1"], (0, 1), (0, 1), (0, 1))
                gate_mms(g_z1, gz1, w_sb["wrz1"], (0, 1), (0, 1), (2, 3))
                bias_mms(g_in1, gaux1, _IN1, (0, 1))
                gate_mms(g_in1, gaux1, w_sb["win1"], (0, 1), (0, 1), (0, 1))
                if t + 1 < nsteps:
                    pend0 = start_l0_groups(t + 1)

                chain(1, g_r1, g_z1, g_hn1, g_in1)  # -> h1(t)

            emit_wout(nsteps - 1)

    _split_waits(nc)
    return nc


def _prep_inputs(encoded_features, step_emb, W_ih0, W_hh0, b_ih0, b_hh0,
                 W_ih1, W_hh1, b_ih1, b_hh1, W_out, b_out):
    """Host-side: slice/shard the big input, transpose + cast weights,
    fold the output projection into layer-0 input weights, fold the
    step-embedding matmul + all additive constants into bias rows."""
    f4 = np.float32
    enc_last = np.asarray(encoded_features)[:, -1].astype(BF16)
    enc_last = np.ascontiguousarray(enc_last)

    W_ih0 = np.asarray(W_ih0, f4)
    W_hh0 = np.asarray(W_hh0, f4)
    W_ih1 = np.asarray(W_ih1, f4)
    W_hh1 = np.asarray(W_hh1, f4)
    W_out = np.asarray(W_out, f4)
    step_emb = np.asarray(step_emb, f4)
    b_ih0 = np.asarray(b_ih0, f4)
    b_hh0 = np.asarray(b_hh0, f4)
    b_ih1 = np.asarray(b_ih1, f4)
    b_hh1 = np.asarray(b_hh1, f4)
    b_out = np.asarray(b_out, f4)

    W_emb = W_ih0[:, :D]          # (768, 256)
    W_pred = W_ih0[:, D:]         # (768, 256)
    W_fold = W_pred @ W_out       # (768, 256): pred feedback folded onto h1
    b_fold = W_pred @ b_out       # (768,)

    # gi_emb[t] = W_emb @ step_emb[t] + b_ih0  -> (12, 768)
    gi_emb = step_emb[:STEPS] @ W_emb.T + b_ih0[None, :]

    def kstack(*mats_cols):
        # (nk, 128, M) k-chunks, then to [128 part, nk, M] contiguous
        chunks = []
        for mat, cols in mats_cols:
            mt = np.ascontiguousarray(mat.T[:, cols])  # (K, M)
            for k in range(0, mt.shape[0], 128):
                chunks.append(mt[k : k + 128])
        arr = np.stack(chunks)                         # (nk, 128, M)
        return np.ascontiguousarray(arr.transpose(1, 0, 2)).astype(BF16)

    rz = slice(0, 512)
    ng = slice(512, 768)
    wrz0 = kstack((W_fold, rz), (W_hh0, rz))          # K: h1c0,h1c1,h0c0,h0c1
    win0 = kstack((W_fold, ng))
    whn0 = kstack((W_hh0, ng))
    wrz1 = kstack((W_ih1, rz), (W_hh1, rz))           # K: h0c0,h0c1,h1c0,h1c1
    whn1 = kstack((W_hh1, ng))
    win1 = kstack((W_ih1, ng))
    wout_s = np.stack([np.ascontiguousarray(W_out.T)[k : k + 128]
                       for k in (0, 128)])            # (2, 128, 256)
    wout_s = np.ascontiguousarray(wout_s.transpose(1, 0, 2)).astype(BF16)
    # single contiguous weight wall [128, 6656]; order matches WOFF
    wall = np.concatenate(
        [a.reshape(128, -1) for a in
         (wrz0, win0, whn0, wrz1, win1, whn1, wout_s)], axis=1)
    wall = np.ascontiguousarray(wall)

    brows = np.zeros(NBROW * 128, f4)

    def put(base, vec):
        brows[base * 128 : base * 128 + len(vec)] = vec

    for t in range(STEPS):
        extra = b_fold if t > 0 else np.zeros_like(b_fold)
        put(_RZ0 + t * 4, gi_emb[t, :512] + b_hh0[:512] + extra[:512])
        put(_IN0 + t * 2, gi_emb[t, 512:] + extra[512:])
    put(_HN0, b_hh0[512:])
    put(_RZ1, b_ih1[:512] + b_hh1[:512])
    put(_IN1, b_ih1[512:])
    put(_HN1, b_hh1[512:])
    put(_BOUT, b_out)
    brows = brows.astype(BF16)[None, :]

    shared = dict(wall=wall, brows=brows)
    in_maps = []
    for i in range(N_CORES):
        m = dict(shared)
        m["enc"] = enc_last[i * PC : (i + 1) * PC]
        in_maps.append(m)
    return in_maps


_CACHE = {}


def _run(in_maps, trace=False):
    from concourse.bass_utils import run_bass_kernel_spmd

    if "nc" not in _CACHE:
        _CACHE["nc"] = build_kernel()
    nc = _CACHE["nc"]
    res = run_bass_kernel_spmd(
        nc, in_maps, core_ids=list(range(N_CORES)), trace=trace
    )
    preds = np.concatenate([res.results[i]["out"] for i in range(N_CORES)],
                           axis=0).astype(np.float32)
    return preds, res


def kernel(encoded_features, step_emb, W_ih0, W_hh0, b_ih0, b_hh0,
           W_ih1, W_hh1, b_ih1, b_hh1, W_out, b_out, num_steps):
    assert int(num_steps) == STEPS
    in_maps = _prep_inputs(encoded_features, step_emb, W_ih0, W_hh0, b_ih0,
                           b_hh0, W_ih1, W_hh1, b_ih1, b_hh1, W_out, b_out)
    preds, _ = _run(in_maps, trace=False)
    return preds


# revision 24
# speedup vs baseline: 1.0757x; 1.0757x over previous
"""Trainium2 Bass kernel for the autoregressive GRU decode head.

Problem: context = mean over zones of encoded_features[:, -1]  -> (B, D)
then 12 autoregressive steps of a 2-layer GRU (H=256) + linear projection
to N=256 zones.  B=1024, data-parallel across 8 NeuronCores (128 batch each).

Key structure (per core, feature-major / "transposed" activations):
  actT (128p, 4 slots, 128) bf16 : [h0 c0, h0 c1, h1 c0, h1 c1]
       slot holds h[b, c*128 + p] at [p, b]    (c = chunk of the 256-dim)
  Gate tensors (PSUM) use layout [p, c*128 + b].
  Matmuls: out(gate_chunk, B) = lhsT.T @ rhs, lhsT = W^T tile (K<=128, M=128),
  rhs = actT slot (K=128, B=128), K-chunks accumulated in PSUM.
  The prediction feedback is algebraically folded into layer 0's weights:
  W_pred @ (W_out h1 + b_out) = (W_pred W_out) h1 + W_pred b_out, so the
  recurrence never materializes pred; pred is computed batch-major only for
  the DRAM output.
  ALL gate biases are injected into PSUM as K=1 matmuls (bias-row x ones),
  so each gate chain is: one wide sigmoid ACT + single-instruction DVE ops.
  Matmul emission is software-pipelined across steps: the h0-dependent
  matmul groups for step t+1 are emitted right after chain(0) of step t, so
  each sigmoid only waits on the 8 matmuls that depend on the newest hidden.
The encoded_features slice is streamed as bf16 (host-converted); the zone
mean is a pairwise add tree split across DVE and GpSimd.
"""

import sys

for _p in ("/opt/trn_rl_repo",):
    if _p not in sys.path:
        sys.path.insert(0, _p)

import numpy as np
import ml_dtypes

import concourse.bass as bass
import concourse.tile as tile
from concourse import mybir
from concourse.vector_clock import ScopedClock

BF16 = ml_dtypes.bfloat16

B, T, NZ, D = 1024, 8, 256, 256
H = 256
STEPS = 12
N_CORES = 8
PC = B // N_CORES  # 128 batch per core

F32 = mybir.dt.float32
BF = mybir.dt.bfloat16
AF = mybir.ActivationFunctionType
OP = mybir.AluOpType

# bias-row column layout in the (1, NBROW*128) bf16 bias-rows tensor:
# value[row*128 + j] is the bias for gate index (chunk c) at position j
_RZ0 = 0                      # 12 steps * 4 chunks
_IN0 = _RZ0 + STEPS * 4      # 12 steps * 2 chunks
_HN0 = _IN0 + STEPS * 2      # 2
_RZ1 = _HN0 + 2               # 4
_IN1 = _RZ1 + 4               # 2
_HN1 = _IN1 + 2               # 2
_BOUT = _HN1 + 2              # 2 (b_out folded into the wout PSUM group)
NBROW = _BOUT + 2             # 84


def _install_tile_drain_patch():
    """walrus (CoreV3) rejects >1 sync wait on the tail drain; spill extras
    onto preceding sync nops."""
    if getattr(tile.TileContext, "_drain_patch_installed", False):
        return

    def _patched(self, tick_clock, wait_clock):
        nc = self.nc
        bb = nc.cur_bb.bb
        drain_bi = nc.sync.drain()
        drain_inst = drain_bi.ins
        wait_clock.add_sem_waits(
            drain_inst, ScopedClock({None: tick_clock.global_clock})
        )
        w = drain_inst.sync_info.on_wait if drain_inst.sync_info else None
        maxw = 1
        if w and len(w) > maxw:
            extra = list(w[maxw:])
            drain_inst.sync_info.on_wait = list(w[:maxw])
            idx = bb.instructions.index(drain_inst)
            nops = []
            for i in range(0, len(extra), maxw):
                nop_bi = nc.sync.nop()
                nop = nop_bi.ins
                si = nop.sync_info
                nop.sync_info = mybir.SyncInfo(
                    on_wait=extra[i : i + maxw],
                    on_update=(si.on_update if si else []),
                )
                bb.instructions.remove(nop)
                nops.append(nop)
            bb.instructions[idx:idx] = nops
        nc.all_engine_barrier()
        popped = nc._tile_sem_poison_stack.pop()
        assert popped is self._sem_poison
        nc.clear_and_free_semaphores(list(self.sems.allocated().values()))
        nc.all_engine_barrier()

    tile.TileContext._drain_and_barrier = _patched
    tile.TileContext._drain_patch_installed = True


def _split_waits(nc, maxw=1):
    """This walrus build rejects instructions carrying more than ~1 sem
    wait; spill extra waits onto same-engine nops placed just before."""
    for bb in nc.main_func.blocks:
        new_list = []
        for inst in bb.instructions:
            si = inst.sync_info
            w = list(si.on_wait) if si and si.on_wait else []
            if len(w) > maxw:
                keep = w[len(w) - maxw:]
                extra = w[: len(w) - maxw]
                si.on_wait = keep
                for i in range(0, len(extra), maxw):
                    nop = mybir.InstNoOp(
                        name=f"{inst.name}-sw{i}", ins=[], outs=[]
                    )
                    nop.engine = inst.engine
                    nop.sync_info = mybir.SyncInfo(
                        on_wait=extra[i : i + maxw], on_update=[]
                    )
                    nc.register_instruction(nop)
                    new_list.append(nop)
            new_list.append(inst)
        bb.instructions[:] = new_list


class _Group:
    """Tracks start/stop flags for a PSUM accumulation group whose matmuls
    are emitted in several program-order batches."""

    def __init__(self, total):
        self.total = total
        self.emitted = 0

    def flags(self):
        start = self.emitted == 0
        self.emitted += 1
        return start, self.emitted == self.total


def build_kernel(nsteps=12):
    """Build the per-core Bass graph (SPMD: same graph on all 8 cores)."""
    _install_tile_drain_patch()
    nc = bass.Bass()

    enc = nc.declare_dram_parameter("enc", [PC, NZ, D], BF, isOutput=False)
    # all matmul weights pre-arranged on host into one contiguous wall
    # [128 part, 6656] so phase 1 needs only ONE weight DMA (the tile
    # framework has an 8-semaphore DMA pool; more DMAs = false serialization)
    WOFF = {"wrz0": (0, 4, 512), "win0": (2048, 2, 256),
            "whn0": (2560, 2, 256), "wrz1": (3072, 4, 512),
            "win1": (5120, 2, 256), "whn1": (5632, 2, 256),
            "wout": (6144, 2, 256)}
    wall = nc.declare_dram_parameter("wall", [128, 6656], BF, isOutput=False)
    brows = nc.declare_dram_parameter("brows", [1, NBROW * 128], BF,
                                      isOutput=False)
    out = nc.declare_dram_parameter("out", [PC, STEPS, NZ], BF, isOutput=True)

    with tile.TileContext(nc) as tc:
        with (
            tc.tile_pool(name="consts", bufs=1) as consts,
            tc.tile_pool(name="state", bufs=1) as state,
            tc.tile_pool(name="enc_pool", bufs=1) as enc_pool,
            tc.tile_pool(name="gates", bufs=2) as gates,
            tc.tile_pool(name="ostage", bufs=4) as ostage,
            tc.tile_pool(name="psum", bufs=1, space="PSUM") as psum,
        ):
            # ---- phase 1 enc DMA: one queue, 64-zone chunks ----
            # Per-engine DMA descriptor latency (~1.1us) floors every
            # 128-partition DMA at ~10us; 64-zone chunks (32KB/descriptor)
            # are HBM-bound (~420 GB/s).  Weights follow ON THE SAME queue
            # so the early HBM bandwidth is all enc.  All chunks resident.
            ZCHS = [64, 64, 64, 64]
            NCH = len(ZCHS)
            e_tiles = []
            z0 = 0
            for ci, zch in enumerate(ZCHS):
                e_sb = enc_pool.tile([128, zch * D], BF, tag=f"echunk{ci}")
                nc.sync.dma_start(e_sb[:], enc[:, z0 : z0 + zch, :])
                e_tiles.append(e_sb)
                z0 += zch
            wall_sb = consts.tile([128, 6656], BF, tag="wall")
            nc.sync.dma_start(wall_sb[:], wall[:])
            w_sb = {
                name: wall_sb[:, off : off + kc * mdim].rearrange(
                    "p (k m) -> p k m", k=kc)
                for name, (off, kc, mdim) in WOFF.items()
            }
            brow_sb = consts.tile([1, NBROW * 128], BF, tag="brow")
            nc.sync.dma_start(brow_sb[:], brows[:])
            ones_row = consts.tile([1, 128], BF, tag="ones")
            nc.gpsimd.memset(ones_row[:], 1.0)

            identity = consts.tile([128, 128], F32, tag="ident")
            nc.gpsimd.memset(identity[:], 0.0)
            nc.gpsimd.affine_select(
                out=identity[:],
                in_=identity[:],
                compare_op=OP.not_equal,
                fill=1.0,
                base=0,
                pattern=[[-1, 128]],
                channel_multiplier=1,
            )
            # prewarm the sigmoid/tanh ACT table during phase 1
            warm = consts.tile([128, 1], F32, tag="warm")
            nc.scalar.activation(warm[:], identity[:, 0:1], AF.Sigmoid)

            # ---- phase 1: zone-mean tree per chunk (all DVE; GpSimd
            # measured ~5 ns/elem - useless).  Each chunk's partial sum is
            # transpose-ACCUMULATED into two PSUM banks (borrowing the
            # r0/z0 decode banks), so no serial merge chain or transpose
            # ladder delays decode start.
            GSHARE = False
            partials = state.tile([128, NCH, D], F32, tag="partials")
            ctpsA = psum.tile([128, 256], F32, tag="r0", bufs=1)
            ctpsB = psum.tile([128, 256], F32, tag="z0", bufs=1)
            gtrA = _Group(NCH)
            gtrB = _Group(NCH)
            for i in range(NCH):
                e_sb = e_tiles[i]
                w = ZCHS[i] * D
                while w > 8 * D:
                    h = w // 2
                    fd = (h - h // 6) // D * D if (GSHARE and h >= 4096) else h
                    nc.vector.tensor_tensor(
                        e_sb[:, 0:fd], e_sb[:, 0:fd], e_sb[:, h : h + fd],
                        OP.add,
                    )
                    if fd < h:
                        nc.gpsimd.tensor_tensor(
                            e_sb[:, fd:h], e_sb[:, fd:h], e_sb[:, h + fd : w],
                            OP.add,
                        )
                    w = h
                # bf16 tail keeps the DVE 2x-mode (f32 only at the end)
                tmp = gates.tile([128, 4 * D], BF, tag="redtail")
                h = w // 2
                nc.vector.tensor_tensor(
                    tmp[:, 0:h], e_sb[:, 0:h], e_sb[:, h:w], OP.add
                )
                w = h
                while w > 2 * D:
                    if (w // 2) % D == 0:
                        h = w // 2
                        nc.vector.tensor_tensor(
                            tmp[:, 0:h], tmp[:, 0:h], tmp[:, h:w], OP.add
                        )
                    else:  # peel one D-block (w = 3*2^k*D cases)
                        h = w - D
                        nc.vector.tensor_tensor(
                            tmp[:, h - D : h], tmp[:, h - D : h], tmp[:, h:w],
                            OP.add,
                        )
                    w = h
                nc.vector.tensor_tensor(
                    partials[:, i, :], tmp[:, 0:D], tmp[:, D : 2 * D], OP.add
                )
                stA, spA = gtrA.flags()
                nc.tensor.matmul(
                    ctpsA[:, 0:128], partials[:, i, 0:128], identity[:],
                    is_transpose=True, start=stA, stop=spA,
                )
                stB, spB = gtrB.flags()
                nc.tensor.matmul(
                    ctpsB[:, 0:128], partials[:, i, 128:256], identity[:],
                    is_transpose=True, start=stB, stop=spB,
                )

            # ---- state: actT slots [h0c0, h0c1, h1c0, h1c1] ----
            # copies split across ACT and DVE so they run in parallel
            actT = state.tile([128, 4, 128], BF, tag="actT")
            for c, ctps in ((0, ctpsA), (1, ctpsB)):
                nc.scalar.activation(actT[:, c, :], ctps[:, 0:128], AF.Copy,
                                     scale=1.0 / NZ)
                nc.vector.tensor_scalar_mul(actT[:, 2 + c, :], ctps[:, 0:128],
                                            1.0 / NZ)

            def bias_mms(g, grp, base, ms):
                # inject bias rows into PSUM: out[c*128+p, b] += brow[p] * 1
                for mi, m in enumerate(ms):
                    st, sp = grp.flags()
                    nc.tensor.matmul(
                        g[:, mi * 128 : (mi + 1) * 128],
                        brow_sb[0:1, (base + m) * 128 : (base + m + 1) * 128],
                        ones_row[:], start=st, stop=sp,
                    )

            def gate_mms(g, grp, w_t, kis, slots, ms):
                for mi, m in enumerate(ms):
                    dst = g[:, mi * 128 : (mi + 1) * 128]
                    for ki, slot in zip(kis, slots):
                        st, sp = grp.flags()
                        nc.tensor.matmul(
                            dst, w_t[:, ki, m * 128 : (m + 1) * 128],
                            actT[:, slot, :], start=st, stop=sp,
                        )

            def chain(layer, g_r, g_z, g_hn, g_in):
                # all biases are already accumulated into the PSUM groups.
                # r and z are SEPARATE tiles so each sigmoid waits only its
                # own 4 matmuls (tile-granular dependency tracking).
                h_sl = (0, 1) if layer == 0 else (2, 3)
                s_ = gates.tile([128, 512], BF, tag=f"S{layer}")
                nc.scalar.activation(s_[:, 0:256], g_r[:], AF.Sigmoid)
                nc.scalar.activation(s_[:, 256:512], g_z[:], AF.Sigmoid)
                t_ = gates.tile([128, 256], F32, tag=f"tt{layer}")
                nc.vector.tensor_tensor(t_[:], g_hn[:], s_[:, 0:256], OP.mult)
                # v = g_in + t  (in-place accumulate into the PSUM tile)
                nc.vector.tensor_tensor(g_in[:], g_in[:], t_[:], OP.add)
                hv = actT[:, h_sl[0] : h_sl[0] + 2, :].rearrange(
                    "p a b -> p (a b)")
                # zm1 = z - 1 off the spine, so q_ is a cheap TT (not STT)
                zm1 = gates.tile([128, 256], BF, tag=f"zm{layer}")
                nc.vector.tensor_scalar_add(zm1[:], s_[:, 256:512], -1.0)
                c_ = gates.tile([128, 256], BF, tag=f"cc{layer}")
                nc.vector.tensor_tensor(c_[:], s_[:, 256:512], hv, OP.mult)
                n_ = gates.tile([128, 256], BF, tag=f"nn{layer}")
                nc.scalar.activation(n_[:], g_in[:], AF.Tanh)
                q_ = gates.tile([128, 256], BF, tag=f"qq{layer}")
                nc.vector.tensor_tensor(q_[:], zm1[:], n_[:], OP.mult)
                nc.vector.tensor_tensor(hv, c_[:], q_[:], OP.subtract)

            def emit_wout(t):
                # output projection, batch-major (off the recurrence)
                g_pb = psum.tile([128, 256], F32, tag="predB", bufs=1)
                nc.tensor.matmul(
                    g_pb[:], ones_row[:],
                    brow_sb[0:1, _BOUT * 128 : _BOUT * 128 + 256],
                    start=True, stop=False,
                )
                for ki, slot in ((0, 2), (1, 3)):
                    nc.tensor.matmul(
                        g_pb[:], actT[:, slot, :], w_sb["wout"][:, ki, :],
                        start=False, stop=(ki == 1),
                    )
                # stage on the scalar engine (GpSimd cannot read PSUM;
                # a DVE placement would head-of-line block the recurrence)
                o_ = ostage.tile([128, 256], BF, tag="ost")
                nc.scalar.activation(o_[:], g_pb[:], AF.Copy)
                nc.sync.dma_start(out[:, t, :], o_[:])

            def start_l0_groups(t):
                """Emit the h0(t-1)-dependent parts of step t's layer-0
                groups (bias rows + hidden matmuls).  The fold (h1) parts
                complete the groups at the top of step t."""
                g_r0 = psum.tile([128, 256], F32, tag="r0", bufs=1)
                g_z0 = psum.tile([128, 256], F32, tag="z0", bufs=1)
                # hn0 and in0 share one PSUM bank (one accumulation group)
                g_aux0 = psum.tile([128, 512], F32, tag="aux0", bufs=1)
                g_hn0 = g_aux0[:, 0:256]
                g_in0 = g_aux0[:, 256:512]
                gr0 = _Group(2 + 4 + (4 if t > 0 else 0))
                gz0 = _Group(2 + 4 + (4 if t > 0 else 0))
                gaux0 = _Group(2 + 4 + 2 + (4 if t > 0 else 0))
                bias_mms(g_r0, gr0, _RZ0 + t * 4, (0, 1))
                gate_mms(g_r0, gr0, w_sb["wrz0"], (2, 3), (0, 1), (0, 1))
                bias_mms(g_z0, gz0, _RZ0 + t * 4, (2, 3))
                gate_mms(g_z0, gz0, w_sb["wrz0"], (2, 3), (0, 1), (2, 3))
                bias_mms(g_hn0, gaux0, _HN0, (0, 1))
                gate_mms(g_hn0, gaux0, w_sb["whn0"], (0, 1), (0, 1), (0, 1))
                bias_mms(g_in0, gaux0, _IN0 + t * 2, (0, 1))
                return g_r0, gr0, g_z0, gz0, g_hn0, g_in0, gaux0

            # ---- phase 2: 12 decode steps, software-pipelined emission ----
            pend0 = start_l0_groups(0)  # h0(-1) = ctx, no fold at t=0

            for t in range(nsteps):
                g_r0, gr0, g_z0, gz0, g_hn0, g_in0, gaux0 = pend0

                # --- phase A: needs h1(t-1) ---
                if t > 0:
                    # pred feedback folded onto h1(t-1); r first (spine)
                    gate_mms(g_r0, gr0, w_sb["wrz0"], (0, 1), (2, 3), (0, 1))
                    gate_mms(g_z0, gz0, w_sb["wrz0"], (0, 1), (2, 3), (2, 3))
                    gate_mms(g_in0, gaux0, w_sb["win0"], (0, 1), (2, 3),
                             (0, 1))
                # layer 1 own-hidden parts (need h1(t-1) only)
                g_r1 = psum.tile([128, 256], F32, tag="r1", bufs=1)
                g_z1 = psum.tile([128, 256], F32, tag="z1", bufs=1)
                # hn1 and in1 share one PSUM bank (one accumulation group)
                g_aux1 = psum.tile([128, 512], F32, tag="aux1", bufs=1)
                g_hn1 = g_aux1[:, 0:256]
                g_in1 = g_aux1[:, 256:512]
                gr1 = _Group(2 + 4 + 4)
                gz1 = _Group(2 + 4 + 4)
                gaux1 = _Group(2 + 4 + 2 + 4)
                bias_mms(g_r1, gr1, _RZ1, (0, 1))
                gate_mms(g_r1, gr1, w_sb["wrz1"], (2, 3), (2, 3), (0, 1))
                bias_mms(g_z1, gz1, _RZ1, (2, 3))
                gate_mms(g_z1, gz1, w_sb["wrz1"], (2, 3), (2, 3), (2, 3))
                bias_mms(g_hn1, gaux1, _HN1, (0, 1))
                gate_mms(g_hn1, gaux1, w_sb["whn1"], (0, 1), (2, 3), (0, 1))
                if t > 0:
                    emit_wout(t - 1)

                chain(0, g_r0, g_z0, g_hn0, g_in0)  # -> h0(t)

                # --- phase B: needs h0(t) ---
                gate_mms(g_r1, gr1, w_sb["wrz1"], (0, 1), (0, 1), (0, 1))
                gate_mms(g_z1, gz1, w_sb["wrz1"], (0, 1), (0, 1), (2, 3))
                bias_mms(g_in1, gaux1, _IN1, (0, 1))
                gate_mms(g_in1, gaux1, w_sb["win1"], (0, 1), (0, 1), (0, 1))
                if t + 1 < nsteps:
                    pend0 = start_l0_groups(t + 1)

                chain(1, g_r1, g_z1, g_hn1, g_in1)  # -> h1(t)

            emit_wout(nsteps - 1)

    _split_waits(nc)
    return nc


def _prep_inputs(encoded_features, step_emb, W_ih0, W_hh0, b_ih0, b_hh0,
                 W_ih1, W_hh1, b_ih1, b_hh1, W_out, b_out):
    """Host-side: slice/shard the big input, transpose + cast weights,
    fold the output projection into layer-0 input weights, fold the
    step-embedding matmul + all additive constants into bias rows."""
    f4 = np.float32
    enc_last = np.asarray(encoded_features)[:, -1].astype(BF16)
    enc_last = np.ascontiguousarray(enc_last)

    W_ih0 = np.asarray(W_ih0, f4)
    W_hh0 = np.asarray(W_hh0, f4)
    W_ih1 = np.asarray(W_ih1, f4)
    W_hh1 = np.asarray(W_hh1, f4)
    W_out = np.asarray(W_out, f4)
    step_emb = np.asarray(step_emb, f4)
    b_ih0 = np.asarray(b_ih0, f4)
    b_hh0 = np.asarray(b_hh0, f4)
    b_ih1 = np.asarray(b_ih1, f4)
    b_hh1 = np.asarray(b_hh1, f4)
    b_out = np.asarray(b_out, f4)

    W_emb = W_ih0[:, :D]          # (768, 256)
    W_pred = W_ih0[:, D:]         # (768, 256)
    W_fold = W_pred @ W_out       # (768, 256): pred feedback folded onto h1
    b_fold = W_pred @ b_out       # (768,)

    # gi_emb[t] = W_emb @ step_emb[t] + b_ih0  -> (12, 768)
    gi_emb = step_emb[:STEPS] @ W_emb.T + b_ih0[None, :]

    def kstack(*mats_cols):
        # (nk, 128, M) k-chunks, then to [128 part, nk, M] contiguous
        chunks = []
        for mat, cols in mats_cols:
            mt = np.ascontiguousarray(mat.T[:, cols])  # (K, M)
            for k in range(0, mt.shape[0], 128):
                chunks.append(mt[k : k + 128])
        arr = np.stack(chunks)                         # (nk, 128, M)
        return np.ascontiguousarray(arr.transpose(1, 0, 2)).astype(BF16)

    rz = slice(0, 512)
    ng = slice(512, 768)
    wrz0 = kstack((W_fold, rz), (W_hh0, rz))          # K: h1c0,h1c1,h0c0,h0c1
    win0 = kstack((W_fold, ng))
    whn0 = kstack((W_hh0, ng))
    wrz1 = kstack((W_ih1, rz), (W_hh1, rz))           # K: h0c0,h0c1,h1c0,h1c1
    whn1 = kstack((W_hh1, ng))
    win1 = kstack((W_ih1, ng))
    wout_s = np.stack([np.ascontiguousarray(W_out.T)[k : k + 128]
                       for k in (0, 128)])            # (2, 128, 256)
    wout_s = np.ascontiguousarray(wout_s.transpose(1, 0, 2)).astype(BF16)
    # single contiguous weight wall [128, 6656]; order matches WOFF
    wall = np.concatenate(
        [a.reshape(128, -1) for a in
         (wrz0, win0, whn0, wrz1, win1, whn1, wout_s)], axis=1)
    wall = np.ascontiguousarray(wall)

    brows = np.zeros(NBROW * 128, f4)

    def put(base, vec):
        brows[base * 128 : base * 128 + len(vec)] = vec

    for t in range(STEPS):
        extra = b_fold if t > 0 else np.zeros_like(b_fold)
        put(_RZ0 + t * 4, gi_emb[t, :512] + b_hh0[:512] + extra[:512])
        put(_IN0 + t * 2, gi_emb[t, 512:] + extra[512:])
    put(_HN0, b_hh0[512:])
    put(_RZ1, b_ih1[:512] + b_hh1[:512])
    put(_IN1, b_ih1[512:])
    put(_HN1, b_hh1[512:])
    put(_BOUT, b_out)
    brows = brows.astype(BF16)[None, :]

    shared = dict(wall=wall, brows=brows)
    in_maps = []
    for i in range(N_CORES):
        m = dict(shared)
        m["enc"] = enc_last[i * PC : (i + 1) * PC]
        in_maps.append(m)
    return in_maps


_CACHE = {}


def _run(in_maps, trace=False):
    from concourse.bass_utils import run_bass_kernel_spmd

    if "nc" not in _CACHE:
        _CACHE["nc"] = build_kernel()
    nc = _CACHE["nc"]
    res = run_bass_kernel_spmd(
        nc, in_maps, core_ids=list(range(N_CORES)), trace=trace
    )
    preds = np.concatenate([res.results[i]["out"] for i in range(N_CORES)],
                           axis=0).astype(np.float32)
    return preds, res


def kernel(encoded_features, step_emb, W_ih0, W_hh0, b_ih0, b_hh0,
           W_ih1, W_hh1, b_ih1, b_hh1, W_out, b_out, num_steps):
    assert int(num_steps) == STEPS
    in_maps = _prep_inputs(encoded_features, step_emb, W_ih0, W_hh0, b_ih0,
                           b_hh0, W_ih1, W_hh1, b_ih1, b_hh1, W_out, b_out)
    preds, _ = _run(in_maps, trace=False)
    return preds
